# revision 31
# baseline (speedup 1.0000x reference)
"""Trainium2 Bass kernel for BatchAll triplet loss.

Reference computation (B=512, D=1024):
    pw = img @ sent.T                                  [B, B]
    t[a,p,n] = pw[a,p] - pw[a,n] + margin
    valid[a,p,n] = (lab[a]==lab[p]) & (lab[a]!=lab[n])
    loss = sum(relu(valid*t)) / (count(valid*t > EPS) + EPS)

Strategy: the batch is class-sorted on the host (a pure permutation of the
(image, sentence, label) triples; the loss is permutation invariant), then
anchors are sharded across 8 cores (64 each, C = core*64). After sorting,
all positives of anchor a live in a contiguous class run inside the core's
128-wide sentence window [C-32, C+96) (holds when max class size <= 33;
dense fallback otherwise). Each core enumerates its actual valid (a,p)
pairs (sum of class sizes over its anchors, ~320 for uniform labels) and
packs them onto partitions: tiles of 128 pairs, free axis = all 512 n.

Per core, with the sentence axis pre-rotated so the window is cols [0,128):
    pw[a,n] (PE, 8 k-tiles)                              [64, 512]
    zext[a,n] = -pw + penM  (penM = -30000 on same-label n)  fp16
    per pair-tile t (T = ceil(maxpairs/128)):
        Z[k,n] = zext[a_k, n]        (PE one-hot broadcast)  [128, 512]
        w[k]   = pw[a_k,p_k]+margin  (DVE: Z's window cols already hold
                 -pw[a_k,p_k]-30000 at col j_k; one-hot dot + init scalar)
        ACT  relu(Z + w) with accum_out row-sums -> Sacc
        DVE  count r > EPS with accum_out        -> Cacc
Host combines the 8 (sum, count) pairs and divides.
"""

import numpy as np
from contextlib import ExitStack

B = 512
D = 1024
NCORES = 8
A = B // NCORES   # 64 anchors per core
KT = D // 128     # 8 contraction tiles
NT = B // 128     # 4 n-tiles per anchor (dense variant)
W = 128           # per-core sentence window width
MARGIN = 0.2
EPS = 1e-16
BIG = 1e30
BIGW = 30000.0
MAXC_WIN = 33     # pair variant valid iff max class size <= this
FP8 = True        # embeddings in fp8e5m2: halves the packT DMA

_CACHE = {}


def _build_pairs(T):
    """Pair-packed kernel: T tiles of 128 (anchor, positive) pairs."""
    import concourse.mybir as mybir
    import concourse.tile as tile
    from concourse import bacc

    f32 = mybir.dt.float32
    f16 = mybir.dt.float16
    Alu = mybir.AluOpType
    Act = mybir.ActivationFunctionType
    Ax = mybir.AxisListType

    nc = bacc.Bacc("TRN2", target_bir_lowering=False, debug=False,
                   num_devices=NCORES)

    K = T * 128
    # aux packs oneJ | selA | penM into one [128, K + K + B] fp16 tensor
    # moved by a single SWDGE transfer (per-transfer completion latency is
    # ~1us, so many small HWDGE transfers would serialize)
    AUXW = 2 * K + B
    f8 = mybir.dt.float8e5
    packT_d = nc.dram_tensor("packT", [D, A + B], f8 if FP8 else f16,
                             kind="ExternalInput")
    aux_d = nc.dram_tensor("aux", [128, AUXW], f16, kind="ExternalInput")
    out_d = nc.dram_tensor("out", [128, 2 * T], f32, kind="ExternalOutput")

    with tile.TileContext(nc) as tc:
        with ExitStack() as ctx:
            singles = ctx.enter_context(tc.tile_pool(name="singles", bufs=1))
            rpool = ctx.enter_context(tc.tile_pool(name="rpool", bufs=3))
            mpool = ctx.enter_context(tc.tile_pool(name="mpool", bufs=3))
            jpool = ctx.enter_context(tc.tile_pool(name="jpool", bufs=2))
            spsum = ctx.enter_context(
                tc.tile_pool(name="spsum", bufs=1, space="PSUM"))
            wpsum = ctx.enter_context(
                tc.tile_pool(name="wpsum", bufs=3, space="PSUM"))
            gpsum = ctx.enter_context(
                tc.tile_pool(name="gpsum", bufs=2, space="PSUM"))

            # ---- small inputs first (gpsimd queue), packT split per
            # k-tile over two queues so PE starts early ----
            aux = singles.tile([128, AUXW], f16)
            nc.gpsimd.dma_start(out=aux, in_=aux_d.ap())
            oneJ = aux[:, 0:K]                       # [128, T*W] t-major
            selA = aux[0:A + 1, K:2 * K]             # [65, K]
            penM = aux[0:A, 2 * K:2 * K + B]         # [64, B]

            # packT in 2 chunks per HW queue: big transfers amortize the
            # per-transfer completion latency while kt0-1 still land early
            packT = singles.tile([128, KT, A + B], f8 if FP8 else f16)
            packT_v = packT_d.ap().rearrange("(t p) m -> p t m", p=128)
            nc.sync.dma_start(out=packT[:, 0:2, :], in_=packT_v[:, 0:2, :])
            nc.scalar.dma_start(out=packT[:, 2:4, :], in_=packT_v[:, 2:4, :])
            nc.sync.dma_start(out=packT[:, 4:6, :], in_=packT_v[:, 4:6, :])
            nc.scalar.dma_start(out=packT[:, 6:8, :], in_=packT_v[:, 6:8, :])
            imgT = packT[:, :, 0:A]
            sentT = packT[:, :, A:A + B]

            # ---- accumulators: one tile, DMA'd out raw (host reduces) ----
            SCacc = singles.tile([128, 2 * T], f32)
            nc.vector.memset(SCacc, 0.0)
            Sacc = SCacc[:, 0:T]
            Cacc = SCacc[:, T:2 * T]
            wcol = singles.tile([128, T], f32)
            thr = singles.tile([128, T], f32)

            # ---- pairwise rows (sentT pre-rotated: window = cols 0..W) ----
            pw_ps = spsum.tile([A, B], f32, tag="sA")
            for kt in range(KT):
                nc.tensor.matmul(pw_ps, lhsT=imgT[:, kt, :], rhs=sentT[:, kt, :],
                                 start=(kt == 0), stop=(kt == KT - 1))

            # clean fp16 copy of the pw window for the wcol gather (the
            # masked zext holds -pw-30000 whose fp16 ulp is 16); on the
            # vector engine so it lands before zext and the g matmuls can
            # beat the z matmuls onto the PE
            pwin = singles.tile([A, W], f16)
            nc.vector.tensor_scalar(pwin, pw_ps[:, 0:W], 0.0, None, Alu.add)

            # ---- z rows: zext[a,n] = margin - pw[a,n] (margin baked into
            # penM on the host); row 64 = -BIGW for padded pairs ----
            zext = singles.tile([A + 1, B], f16)
            nc.vector.memset(zext[A:A + 1, :], -BIGW)
            nc.vector.scalar_tensor_tensor(zext[0:A, :], pw_ps, -1.0, penM,
                                           Alu.mult, Alu.add)

            # ---- wcol gathers (g matmuls emitted before the z matmuls;
            # the fused mul+rowsum runs on gpsimd, off the vector queue) ----
            for t in range(T):
                sl = selA[:, t * 128:(t + 1) * 128]
                g_ps = gpsum.tile([128, W], f32)
                nc.tensor.matmul(g_ps, lhsT=sl[0:A, :], rhs=pwin)
                # wcol[k] = pw[a_k, p_k]
                junk = jpool.tile([128, W], f16)
                nc.vector.scalar_tensor_tensor(
                    junk, g_ps, 1.0, oneJ[:, t * W:(t + 1) * W],
                    Alu.mult, Alu.mult,
                    accum_out=wcol[:, t:t + 1])
                # count threshold: t > EPS  <=>  z > EPS - wcol
                nc.vector.tensor_scalar(thr[:, t:t + 1], wcol[:, t:t + 1],
                                        -1.0, EPS, Alu.mult, Alu.add)

            # ---- main loop: one tile of 128 pairs per iteration ----
            for t in range(T):
                sl = selA[:, t * 128:(t + 1) * 128]
                z_ps = wpsum.tile([128, B], f32)
                nc.tensor.matmul(z_ps, lhsT=sl, rhs=zext)
                r = rpool.tile([128, B], f16)
                nc.scalar.activation(
                    out=r, in_=z_ps, func=Act.Relu,
                    bias=wcol[:, t:t + 1], scale=1.0,
                    accum_out=Sacc[:, t:t + 1])
                # count reads z_ps directly so it runs concurrently with
                # the relu instead of waiting for r
                m = mpool.tile([128, B], f16)
                nc.vector.tensor_scalar(
                    m, z_ps, thr[:, t:t + 1], None, Alu.is_gt, Alu.add,
                    accum_out=Cacc[:, t:t + 1])

            # ---- ship raw accumulators; host does the final reduction ----
            nc.sync.dma_start(out=out_d.ap(), in_=SCacc)

    nc.compile()
    return nc


def _build_dense():
    """Dense fallback (no class-size assumption)."""
    import concourse.mybir as mybir
    import concourse.tile as tile
    from concourse import bacc
    from concourse.masks import make_identity

    f32 = mybir.dt.float32
    bf16 = mybir.dt.bfloat16
    Alu = mybir.AluOpType
    Act = mybir.ActivationFunctionType
    Ax = mybir.AxisListType

    nc = bacc.Bacc("TRN2", target_bir_lowering=False, debug=False,
                   num_devices=NCORES)

    imgT_d = nc.dram_tensor("imgT", [D, A], f32, kind="ExternalInput")
    sentT_d = nc.dram_tensor("sentT", [D, B], f32, kind="ExternalInput")
    labf_d = nc.dram_tensor("labf", [B], bf16, kind="ExternalInput")
    labc_d = nc.dram_tensor("labc", [A], f32, kind="ExternalInput")
    out_d = nc.dram_tensor("out", [2], f32, kind="ExternalOutput")

    with tile.TileContext(nc) as tc:
        with ExitStack() as ctx:
            singles = ctx.enter_context(tc.tile_pool(name="singles", bufs=1))
            rpool = ctx.enter_context(tc.tile_pool(name="rpool", bufs=6))
            mpool = ctx.enter_context(tc.tile_pool(name="mpool", bufs=6))
            spsum = ctx.enter_context(
                tc.tile_pool(name="spsum", bufs=1, space="PSUM"))
            wpsum = ctx.enter_context(
                tc.tile_pool(name="wpsum", bufs=3, space="PSUM"))

            ones_r = singles.tile([1, 128], f32)
            nc.vector.memset(ones_r, 1.0)
            ones_c = singles.tile([128, 1], f32)
            nc.vector.memset(ones_c, 1.0)
            ident = singles.tile([64, 64], f32)
            make_identity(nc, ident)

            imgT = singles.tile([128, KT, A], f32)
            nc.sync.dma_start(
                out=imgT, in_=imgT_d.ap().rearrange("(t p) m -> p t m", p=128))
            sentT = singles.tile([128, KT, B], f32)
            nc.sync.dma_start(
                out=sentT, in_=sentT_d.ap().rearrange("(t p) m -> p t m", p=128))
            lab_row = singles.tile([1, B], f32)
            nc.sync.dma_start(
                out=lab_row, in_=labf_d.ap().rearrange("(o b) -> o b", o=1))
            labc_col = singles.tile([A, 1], f32)
            nc.sync.dma_start(
                out=labc_col, in_=labc_d.ap().rearrange("(a o) -> a o", o=1))

            pw_ps = spsum.tile([A, B], f32)
            for kt in range(KT):
                nc.tensor.matmul(pw_ps, lhsT=imgT[:, kt, :], rhs=sentT[:, kt, :],
                                 start=(kt == 0), stop=(kt == KT - 1))

            labB_ps = spsum.tile([A, B], f32)
            nc.tensor.matmul(labB_ps, lhsT=ones_r[:, :A], rhs=lab_row)
            eqP = singles.tile([A, B], f32)
            nc.vector.tensor_scalar(eqP, labB_ps, labc_col, None, Alu.is_equal)
            penP = singles.tile([A, B], f32)
            nc.vector.tensor_scalar(penP, eqP, 1.0, BIG, Alu.subtract, Alu.mult)
            penN = singles.tile([A, B], f32)
            nc.vector.tensor_scalar(penN, eqP, -BIG, None, Alu.mult)

            w = singles.tile([A, B], f32)
            nc.vector.tensor_scalar(w, pw_ps, MARGIN, None, Alu.add)
            nc.vector.tensor_mul(w, w, eqP)
            nc.vector.tensor_add(w, w, penP)
            negneq = singles.tile([A, B], f32)
            nc.vector.tensor_scalar(negneq, eqP, 1.0, -1.0, Alu.subtract,
                                    Alu.mult)
            z = singles.tile([A, B], f32)
            nc.vector.tensor_scalar(z, pw_ps, -1.0, None, Alu.mult)
            nc.vector.tensor_mul(z, z, negneq)
            nc.vector.tensor_add(z, z, penN)

            zTs = singles.tile([128, NT, A], f32)
            for j in range(NT):
                zt_ps = spsum.tile([128, A], f32)
                nc.tensor.transpose(zt_ps, z[:, j * 128:(j + 1) * 128], ident)
                nc.scalar.copy(zTs[:, j, :], zt_ps)

            Sacc = singles.tile([128, A * NT], f32)
            Cacc = singles.tile([128, A * NT], f32)

            for a in range(A):
                wb_ps = wpsum.tile([128, B], f32)
                nc.tensor.matmul(
                    wb_ps, lhsT=ident[:, a:a + 1].broadcast_to([A, 128]), rhs=w)
                for j in range(NT):
                    col = a * NT + j
                    r = rpool.tile([128, B], bf16)
                    nc.scalar.activation(
                        out=r, in_=wb_ps, func=Act.Relu,
                        bias=zTs[:, j, a:a + 1], scale=1.0,
                        accum_out=Sacc[:, col:col + 1])
                    m = mpool.tile([128, B], bf16)
                    nc.vector.tensor_scalar(
                        m, r, EPS, None, Alu.is_gt, Alu.add,
                        accum_out=Cacc[:, col:col + 1])

            SC = singles.tile([128, 2], f32)
            nc.vector.tensor_reduce(SC[:, 0:1], Sacc, Ax.X, Alu.add)
            nc.vector.tensor_reduce(SC[:, 1:2], Cacc, Ax.X, Alu.add)
            fin_ps = spsum.tile([2, 1], f32)
            nc.tensor.matmul(fin_ps, lhsT=SC, rhs=ones_c)
            fin_sb = singles.tile([2, 1], f32)
            nc.scalar.copy(fin_sb, fin_ps)
            nc.sync.dma_start(
                out=out_d.ap().rearrange("(p o) -> p o", o=1), in_=fin_sb)

    nc.compile()
    return nc


def _get_nc(variant, T=0):
    key = f"nc_{variant}_{T}"
    if key not in _CACHE:
        _CACHE[key] = (_build_pairs(T) if variant == "pairs"
                       else _build_dense())
    return _CACHE[key]


def _prep(labels, image_embeddings, sentence_embeddings):
    """Class-sort the batch; build per-core input maps."""
    labels = np.ascontiguousarray(labels).astype(np.int64)
    img = np.ascontiguousarray(image_embeddings, dtype=np.float32)
    sent = np.ascontiguousarray(sentence_embeddings, dtype=np.float32)
    counts = np.bincount(labels, minlength=1)
    maxc = counts.max()

    perm = np.argsort(labels, kind="stable")
    labs = labels[perm]

    if maxc > MAXC_WIN:
        imgT = np.ascontiguousarray(img[perm].T)    # [D, B]
        sentT = np.ascontiguousarray(sent[perm].T)  # [D, B]
        labsf = labs.astype(np.float32)
        maps = []
        for i in range(NCORES):
            c0 = i * A
            maps.append({
                "imgT": np.ascontiguousarray(imgT[:, c0:c0 + A]),
                "sentT": sentT,
                "labf": labsf,
                "labc": np.ascontiguousarray(labsf[c0:c0 + A]),
            })
        return "dense", 0, maps

    if FP8:
        import ml_dtypes
        edt = ml_dtypes.float8_e5m2
    else:
        edt = np.float16
    imgT = np.ascontiguousarray(img[perm].T).astype(edt)
    sentT = np.ascontiguousarray(sent[perm].T).astype(edt)

    # class run start/size per sorted position
    starts = np.concatenate([[0], np.cumsum(counts)])
    s_a = starts[labs]            # run start of each anchor
    n_a = counts[labs]            # run length of each anchor
    maxK = max(int(n_a[c0:c0 + A].sum()) for c0 in range(0, B, A))
    T = (maxK + 127) // 128

    maps = []
    for i in range(NCORES):
        c0 = i * A
        rot = (np.arange(B) + c0 - 32) % B
        packT = np.ascontiguousarray(
            np.concatenate([imgT[:, c0:c0 + A], sentT[:, rot]], axis=1))
        # aux layout: [128, K] oneJ | [65, K] selA | [64, B] penM
        K = T * 128
        aux = np.zeros((128, 2 * K + B), np.float16)
        # penM[a, n] = margin, or margin-BIGW where rotated label n
        # matches the anchor label (so zext = margin - pw, masked)
        eq = labs[rot][None, :] == labs[c0:c0 + A][:, None]
        aux[0:A, 2 * K:2 * K + B] = np.where(
            eq, np.float16(MARGIN - BIGW), np.float16(MARGIN))
        # pair list: for each local anchor a, all p in its class run
        k = 0
        for a in range(A):
            ga = c0 + a
            for p in range(int(s_a[ga]), int(s_a[ga] + n_a[ga])):
                j = p - (c0 - 32)
                aux[a, K + k] = 1.0                      # selA
                aux[k % 128, (k // 128) * W + j] = 1.0   # oneJ
                k += 1
        aux[A, K + k:2 * K] = 1.0   # pads select zext row 64 (-BIGW)
        maps.append({"packT": packT, "aux": aux})
    return "pairs", T, maps


def run_all(labels, image_embeddings, sentence_embeddings, trace=False):
    from concourse.bass_utils import run_bass_kernel_spmd
    variant, T, maps = _prep(labels, image_embeddings, sentence_embeddings)
    nc = _get_nc(variant, T)
    res = run_bass_kernel_spmd(nc, maps, list(range(NCORES)), trace=trace)
    parts = np.stack([res.results[i]["out"] for i in range(NCORES)])
    if variant == "pairs":
        s = float(parts[:, :, 0:T].sum())
        c = float(parts[:, :, T:2 * T].sum())
    else:
        s = float(parts[:, 0].sum())
        c = float(parts[:, 1].sum())
    loss = np.float32(s / (c + EPS))
    return np.asarray(loss, dtype=np.float32), res


def kernel(labels, image_embeddings, sentence_embeddings):
    out, _ = run_all(labels, image_embeddings, sentence_embeddings)
    return out


# revision 33
# speedup vs baseline: 1.1207x; 1.1207x over previous
"""Trainium2 Bass kernel for BatchAll triplet loss.

Reference computation (B=512, D=1024):
    pw = img @ sent.T                                  [B, B]
    t[a,p,n] = pw[a,p] - pw[a,n] + margin
    valid[a,p,n] = (lab[a]==lab[p]) & (lab[a]!=lab[n])
    loss = sum(relu(valid*t)) / (count(valid*t > EPS) + EPS)

Strategy: the batch is class-sorted on the host (a pure permutation of the
(image, sentence, label) triples; the loss is permutation invariant), then
anchors are sharded across 8 cores (64 each, C = core*64). After sorting,
all positives of anchor a live in a contiguous class run inside the core's
128-wide sentence window [C-32, C+96) (holds when max class size <= 33;
dense fallback otherwise). Each core enumerates its actual valid (a,p)
pairs (sum of class sizes over its anchors, ~320 for uniform labels) and
packs them onto partitions: tiles of 128 pairs, free axis = all 512 n.

Per core, with the sentence axis pre-rotated so the window is cols [0,128):
    pw[a,n] (PE, 8 k-tiles)                              [64, 512]
    zext[a,n] = -pw + penM  (penM = -30000 on same-label n)  fp16
    per pair-tile t (T = ceil(maxpairs/128)):
        Z[k,n] = zext[a_k, n]        (PE one-hot broadcast)  [128, 512]
        w[k]   = pw[a_k,p_k]+margin  (DVE: Z's window cols already hold
                 -pw[a_k,p_k]-30000 at col j_k; one-hot dot + init scalar)
        ACT  relu(Z + w) with accum_out row-sums -> Sacc
        DVE  count r > EPS with accum_out        -> Cacc
Host combines the 8 (sum, count) pairs and divides.
"""

import numpy as np
from contextlib import ExitStack

B = 512
D = 1024
NCORES = 8
A = B // NCORES   # 64 anchors per core
KT = D // 128     # 8 contraction tiles
NT = B // 128     # 4 n-tiles per anchor (dense variant)
W = 128           # per-core sentence window width
MARGIN = 0.2
EPS = 1e-16
BIG = 1e30
BIGW = 30000.0
MAXC_WIN = 33     # pair variant valid iff max class size <= this
FP8 = True        # embeddings in fp8e5m2: halves the packT DMA

_CACHE = {}


def _build_pairs(T):
    """Pair-packed kernel: T tiles of 128 (anchor, positive) pairs."""
    import concourse.mybir as mybir
    import concourse.tile as tile
    from concourse import bacc

    f32 = mybir.dt.float32
    f16 = mybir.dt.float16
    Alu = mybir.AluOpType
    Act = mybir.ActivationFunctionType
    Ax = mybir.AxisListType

    nc = bacc.Bacc("TRN2", target_bir_lowering=False, debug=False,
                   num_devices=NCORES)

    K = T * 128
    # aux packs oneJ | selA | penM into one [128, K + K + B] fp16 tensor
    # moved by a single SWDGE transfer (per-transfer completion latency is
    # ~1us, so many small HWDGE transfers would serialize)
    AUXW = 2 * K + B
    f8 = mybir.dt.float8e5
    packT_d = nc.dram_tensor("packT", [D, A + B], f8 if FP8 else f16,
                             kind="ExternalInput")
    aux_d = nc.dram_tensor("aux", [128, AUXW], f16, kind="ExternalInput")
    out_d = nc.dram_tensor("out", [128, 2 * T], f32, kind="ExternalOutput")

    with tile.TileContext(nc) as tc:
        with ExitStack() as ctx:
            singles = ctx.enter_context(tc.tile_pool(name="singles", bufs=1))
            rpool = ctx.enter_context(tc.tile_pool(name="rpool", bufs=3))
            mpool = ctx.enter_context(tc.tile_pool(name="mpool", bufs=3))
            jpool = ctx.enter_context(tc.tile_pool(name="jpool", bufs=2))
            spsum = ctx.enter_context(
                tc.tile_pool(name="spsum", bufs=1, space="PSUM"))
            wpsum = ctx.enter_context(
                tc.tile_pool(name="wpsum", bufs=3, space="PSUM"))
            gpsum = ctx.enter_context(
                tc.tile_pool(name="gpsum", bufs=2, space="PSUM"))

            # ---- small inputs first (gpsimd queue), packT split per
            # k-tile over two queues so PE starts early ----
            aux = singles.tile([128, AUXW], f16)
            nc.gpsimd.dma_start(out=aux, in_=aux_d.ap())
            oneJ = aux[:, 0:K]                       # [128, T*W] t-major
            selA = aux[0:A + 1, K:2 * K]             # [65, K]
            penM = aux[0:A, 2 * K:2 * K + B]         # [64, B]

            # packT in ONE transfer per HW queue: per-transfer completion
            # latency is ~2.5-3us regardless of size, so serial transfers
            # on a queue cost far more than their wire time
            packT = singles.tile([128, KT, A + B], f8 if FP8 else f16)
            packT_v = packT_d.ap().rearrange("(t p) m -> p t m", p=128)
            nc.sync.dma_start(out=packT[:, 0:4, :], in_=packT_v[:, 0:4, :])
            nc.scalar.dma_start(out=packT[:, 4:8, :], in_=packT_v[:, 4:8, :])
            imgT = packT[:, :, 0:A]
            sentT = packT[:, :, A:A + B]

            # ---- accumulators: one tile, DMA'd out raw (host reduces) ----
            SCacc = singles.tile([128, 2 * T], f32)
            nc.vector.memset(SCacc, 0.0)
            Sacc = SCacc[:, 0:T]
            Cacc = SCacc[:, T:2 * T]
            wcol = singles.tile([128, T], f32)
            thr = singles.tile([128, T], f32)

            # ---- pairwise rows (sentT pre-rotated: window = cols 0..W).
            # fp8 DoubleRow folds two k-tiles into each matmul ----
            pw_ps = spsum.tile([A, B], f32, tag="sA")
            if FP8:
                for u in range(KT // 2):
                    nc.tensor.matmul(
                        pw_ps, lhsT=imgT[:, 2 * u:2 * u + 2, :],
                        rhs=sentT[:, 2 * u:2 * u + 2, :],
                        start=(u == 0), stop=(u == KT // 2 - 1),
                        perf_mode=mybir.MatmulPerfMode.DoubleRow)
            else:
                for kt in range(KT):
                    nc.tensor.matmul(pw_ps, lhsT=imgT[:, kt, :],
                                     rhs=sentT[:, kt, :],
                                     start=(kt == 0), stop=(kt == KT - 1))

            # clean fp16 copy of the pw window for the wcol gather (the
            # masked zext holds -pw-30000 whose fp16 ulp is 16); on the
            # vector engine so it lands before zext and the g matmuls can
            # beat the z matmuls onto the PE
            pwin = singles.tile([A, W], f16)
            nc.vector.tensor_scalar(pwin, pw_ps[:, 0:W], 0.0, None, Alu.add)

            # ---- z rows: zext[a,n] = margin - pw[a,n] (margin baked into
            # penM on the host); row 64 = -BIGW for padded pairs ----
            zext = singles.tile([A + 1, B], f16)
            nc.vector.memset(zext[A:A + 1, :], -BIGW)
            nc.vector.scalar_tensor_tensor(zext[0:A, :], pw_ps, -1.0, penM,
                                           Alu.mult, Alu.add)

            # ---- wcol gathers (g matmuls emitted before the z matmuls;
            # the fused mul+rowsum runs on gpsimd, off the vector queue) ----
            for t in range(T):
                sl = selA[:, t * 128:(t + 1) * 128]
                g_ps = gpsum.tile([128, W], f32)
                nc.tensor.matmul(g_ps, lhsT=sl[0:A, :], rhs=pwin)
                # wcol[k] = pw[a_k, p_k]
                junk = jpool.tile([128, W], f16)
                nc.vector.scalar_tensor_tensor(
                    junk, g_ps, 1.0, oneJ[:, t * W:(t + 1) * W],
                    Alu.mult, Alu.mult,
                    accum_out=wcol[:, t:t + 1])
                # count threshold: t > EPS  <=>  z > EPS - wcol
                nc.vector.tensor_scalar(thr[:, t:t + 1], wcol[:, t:t + 1],
                                        -1.0, EPS, Alu.mult, Alu.add)

            # ---- main loop: one tile of 128 pairs per iteration ----
            for t in range(T):
                sl = selA[:, t * 128:(t + 1) * 128]
                z_ps = wpsum.tile([128, B], f32)
                nc.tensor.matmul(z_ps, lhsT=sl, rhs=zext)
                r = rpool.tile([128, B], f16)
                nc.scalar.activation(
                    out=r, in_=z_ps, func=Act.Relu,
                    bias=wcol[:, t:t + 1], scale=1.0,
                    accum_out=Sacc[:, t:t + 1])
                # count reads z_ps directly so it runs concurrently with
                # the relu instead of waiting for r
                m = mpool.tile([128, B], f16)
                nc.vector.tensor_scalar(
                    m, z_ps, thr[:, t:t + 1], None, Alu.is_gt, Alu.add,
                    accum_out=Cacc[:, t:t + 1])

            # ---- ship raw accumulators; host does the final reduction ----
            nc.sync.dma_start(out=out_d.ap(), in_=SCacc)

    nc.compile()
    return nc


def _build_dense():
    """Dense fallback (no class-size assumption)."""
    import concourse.mybir as mybir
    import concourse.tile as tile
    from concourse import bacc
    from concourse.masks import make_identity

    f32 = mybir.dt.float32
    bf16 = mybir.dt.bfloat16
    Alu = mybir.AluOpType
    Act = mybir.ActivationFunctionType
    Ax = mybir.AxisListType

    nc = bacc.Bacc("TRN2", target_bir_lowering=False, debug=False,
                   num_devices=NCORES)

    imgT_d = nc.dram_tensor("imgT", [D, A], f32, kind="ExternalInput")
    sentT_d = nc.dram_tensor("sentT", [D, B], f32, kind="ExternalInput")
    labf_d = nc.dram_tensor("labf", [B], bf16, kind="ExternalInput")
    labc_d = nc.dram_tensor("labc", [A], f32, kind="ExternalInput")
    out_d = nc.dram_tensor("out", [2], f32, kind="ExternalOutput")

    with tile.TileContext(nc) as tc:
        with ExitStack() as ctx:
            singles = ctx.enter_context(tc.tile_pool(name="singles", bufs=1))
            rpool = ctx.enter_context(tc.tile_pool(name="rpool", bufs=6))
            mpool = ctx.enter_context(tc.tile_pool(name="mpool", bufs=6))
            spsum = ctx.enter_context(
                tc.tile_pool(name="spsum", bufs=1, space="PSUM"))
            wpsum = ctx.enter_context(
                tc.tile_pool(name="wpsum", bufs=3, space="PSUM"))

            ones_r = singles.tile([1, 128], f32)
            nc.vector.memset(ones_r, 1.0)
            ones_c = singles.tile([128, 1], f32)
            nc.vector.memset(ones_c, 1.0)
            ident = singles.tile([64, 64], f32)
            make_identity(nc, ident)

            imgT = singles.tile([128, KT, A], f32)
            nc.sync.dma_start(
                out=imgT, in_=imgT_d.ap().rearrange("(t p) m -> p t m", p=128))
            sentT = singles.tile([128, KT, B], f32)
            nc.sync.dma_start(
                out=sentT, in_=sentT_d.ap().rearrange("(t p) m -> p t m", p=128))
            lab_row = singles.tile([1, B], f32)
            nc.sync.dma_start(
                out=lab_row, in_=labf_d.ap().rearrange("(o b) -> o b", o=1))
            labc_col = singles.tile([A, 1], f32)
            nc.sync.dma_start(
                out=labc_col, in_=labc_d.ap().rearrange("(a o) -> a o", o=1))

            pw_ps = spsum.tile([A, B], f32)
            for kt in range(KT):
                nc.tensor.matmul(pw_ps, lhsT=imgT[:, kt, :], rhs=sentT[:, kt, :],
                                 start=(kt == 0), stop=(kt == KT - 1))

            labB_ps = spsum.tile([A, B], f32)
            nc.tensor.matmul(labB_ps, lhsT=ones_r[:, :A], rhs=lab_row)
            eqP = singles.tile([A, B], f32)
            nc.vector.tensor_scalar(eqP, labB_ps, labc_col, None, Alu.is_equal)
            penP = singles.tile([A, B], f32)
            nc.vector.tensor_scalar(penP, eqP, 1.0, BIG, Alu.subtract, Alu.mult)
            penN = singles.tile([A, B], f32)
            nc.vector.tensor_scalar(penN, eqP, -BIG, None, Alu.mult)

            w = singles.tile([A, B], f32)
            nc.vector.tensor_scalar(w, pw_ps, MARGIN, None, Alu.add)
            nc.vector.tensor_mul(w, w, eqP)
            nc.vector.tensor_add(w, w, penP)
            negneq = singles.tile([A, B], f32)
            nc.vector.tensor_scalar(negneq, eqP, 1.0, -1.0, Alu.subtract,
                                    Alu.mult)
            z = singles.tile([A, B], f32)
            nc.vector.tensor_scalar(z, pw_ps, -1.0, None, Alu.mult)
            nc.vector.tensor_mul(z, z, negneq)
            nc.vector.tensor_add(z, z, penN)

            zTs = singles.tile([128, NT, A], f32)
            for j in range(NT):
                zt_ps = spsum.tile([128, A], f32)
                nc.tensor.transpose(zt_ps, z[:, j * 128:(j + 1) * 128], ident)
                nc.scalar.copy(zTs[:, j, :], zt_ps)

            Sacc = singles.tile([128, A * NT], f32)
            Cacc = singles.tile([128, A * NT], f32)

            for a in range(A):
                wb_ps = wpsum.tile([128, B], f32)
                nc.tensor.matmul(
                    wb_ps, lhsT=ident[:, a:a + 1].broadcast_to([A, 128]), rhs=w)
                for j in range(NT):
                    col = a * NT + j
                    r = rpool.tile([128, B], bf16)
                    nc.scalar.activation(
                        out=r, in_=wb_ps, func=Act.Relu,
                        bias=zTs[:, j, a:a + 1], scale=1.0,
                        accum_out=Sacc[:, col:col + 1])
                    m = mpool.tile([128, B], bf16)
                    nc.vector.tensor_scalar(
                        m, r, EPS, None, Alu.is_gt, Alu.add,
                        accum_out=Cacc[:, col:col + 1])

            SC = singles.tile([128, 2], f32)
            nc.vector.tensor_reduce(SC[:, 0:1], Sacc, Ax.X, Alu.add)
            nc.vector.tensor_reduce(SC[:, 1:2], Cacc, Ax.X, Alu.add)
            fin_ps = spsum.tile([2, 1], f32)
            nc.tensor.matmul(fin_ps, lhsT=SC, rhs=ones_c)
            fin_sb = singles.tile([2, 1], f32)
            nc.scalar.copy(fin_sb, fin_ps)
            nc.sync.dma_start(
                out=out_d.ap().rearrange("(p o) -> p o", o=1), in_=fin_sb)

    nc.compile()
    return nc


def _get_nc(variant, T=0):
    key = f"nc_{variant}_{T}"
    if key not in _CACHE:
        _CACHE[key] = (_build_pairs(T) if variant == "pairs"
                       else _build_dense())
    return _CACHE[key]


def _prep(labels, image_embeddings, sentence_embeddings):
    """Class-sort the batch; build per-core input maps."""
    labels = np.ascontiguousarray(labels).astype(np.int64)
    img = np.ascontiguousarray(image_embeddings, dtype=np.float32)
    sent = np.ascontiguousarray(sentence_embeddings, dtype=np.float32)
    counts = np.bincount(labels, minlength=1)
    maxc = counts.max()

    perm = np.argsort(labels, kind="stable")
    labs = labels[perm]

    if maxc > MAXC_WIN:
        imgT = np.ascontiguousarray(img[perm].T)    # [D, B]
        sentT = np.ascontiguousarray(sent[perm].T)  # [D, B]
        labsf = labs.astype(np.float32)
        maps = []
        for i in range(NCORES):
            c0 = i * A
            maps.append({
                "imgT": np.ascontiguousarray(imgT[:, c0:c0 + A]),
                "sentT": sentT,
                "labf": labsf,
                "labc": np.ascontiguousarray(labsf[c0:c0 + A]),
            })
        return "dense", 0, maps

    if FP8:
        import ml_dtypes
        edt = ml_dtypes.float8_e5m2
    else:
        edt = np.float16
    imgT = np.ascontiguousarray(img[perm].T).astype(edt)
    sentT = np.ascontiguousarray(sent[perm].T).astype(edt)

    # class run start/size per sorted position
    starts = np.concatenate([[0], np.cumsum(counts)])
    s_a = starts[labs]            # run start of each anchor
    n_a = counts[labs]            # run length of each anchor
    maxK = max(int(n_a[c0:c0 + A].sum()) for c0 in range(0, B, A))
    T = (maxK + 127) // 128

    maps = []
    for i in range(NCORES):
        c0 = i * A
        rot = (np.arange(B) + c0 - 32) % B
        packT = np.ascontiguousarray(
            np.concatenate([imgT[:, c0:c0 + A], sentT[:, rot]], axis=1))
        # aux layout: [128, K] oneJ | [65, K] selA | [64, B] penM
        K = T * 128
        aux = np.zeros((128, 2 * K + B), np.float16)
        # penM[a, n] = margin, or margin-BIGW where rotated label n
        # matches the anchor label (so zext = margin - pw, masked)
        eq = labs[rot][None, :] == labs[c0:c0 + A][:, None]
        aux[0:A, 2 * K:2 * K + B] = np.where(
            eq, np.float16(MARGIN - BIGW), np.float16(MARGIN))
        # pair list: for each local anchor a, all p in its class run
        k = 0
        for a in range(A):
            ga = c0 + a
            for p in range(int(s_a[ga]), int(s_a[ga] + n_a[ga])):
                j = p - (c0 - 32)
                aux[a, K + k] = 1.0                      # selA
                aux[k % 128, (k // 128) * W + j] = 1.0   # oneJ
                k += 1
        aux[A, K + k:2 * K] = 1.0   # pads select zext row 64 (-BIGW)
        maps.append({"packT": packT, "aux": aux})
    return "pairs", T, maps


def run_all(labels, image_embeddings, sentence_embeddings, trace=False):
    from concourse.bass_utils import run_bass_kernel_spmd
    variant, T, maps = _prep(labels, image_embeddings, sentence_embeddings)
    nc = _get_nc(variant, T)
    res = run_bass_kernel_spmd(nc, maps, list(range(NCORES)), trace=trace)
    parts = np.stack([res.results[i]["out"] for i in range(NCORES)])
    if variant == "pairs":
        s = float(parts[:, :, 0:T].sum())
        c = float(parts[:, :, T:2 * T].sum())
    else:
        s = float(parts[:, 0].sum())
        c = float(parts[:, 1].sum())
    loss = np.float32(s / (c + EPS))
    return np.asarray(loss, dtype=np.float32), res


def kernel(labels, image_embeddings, sentence_embeddings):
    out, _ = run_all(labels, image_embeddings, sentence_embeddings)
    return out
